# revision 18
# baseline (speedup 1.0000x reference)
"""Graph attention (BatchedAttentionLayer) Bass kernel for 8 trn2 NeuronCores.

Full-input contract: kernel(**inputs) -> [50000, 8, 16] float32.

Strategy (sharded by destination node):
  - 8 cores x 6250 dst nodes; edges routed to the core owning their dst,
    sorted by dst into 49 windows of 128 dst slots, tiled in 128-edge tiles.
  - Per-core node permutation puts own nodes first so the SPMD program is
    identical across cores.
  - Device prep (bf16): KVQ table [50048, 384] = [h@Wk+bk | h@Wv+bv | h@Wq+bq]
    via TensorE (h_T chunks stationary, bias via ones-row matmul into the
    same PSUM accumulation group).
  - Main: dma_gather pulls K|V rows (512B bf16, int16 idx => table split at
    row 32768); Q comes from per-window table loads expanded per tile with a
    one-hot matmul (no per-edge Q gather).  DVE: K*Q, segmented head-reduce,
    fused clip; ACT: exp(0.25*raw), s head-broadcast; DVE: V*s; TensorE: one
    fused scatter matmul per tile (rhs = [wV | s]) accumulating out+z in one
    PSUM bank per window; reciprocal-divide; DMA out.
"""

import os

import numpy as np
import ml_dtypes

import concourse.bacc as bacc
import concourse.bass as bass
import concourse.mybir as mybir
import concourse.tile as tile
from concourse import library_config
from concourse.bass_utils import run_bass_kernel_spmd

N_NODES = 50000
N_EDGES = 800000
F = 128            # feature dim = H*D
H = 8
D = 16
CORES = 8
NPC = N_NODES // CORES           # 6250 nodes per core
WIN = 128                        # dst nodes per window
NWIN = (NPC + WIN - 1) // WIN    # 49 windows per core
SPLIT = 32768                    # int16-safe KV table split row
KVQ_W = F * 3                    # 384: K | V | Q columns
PREP_TILES = (N_NODES + 127) // 128   # 391
PREP_GROUP = 4                   # node tiles per prep DMA group
Q_ROWS = NWIN * WIN              # 6272
SB_WINDOWS = 3                   # windows per gather super-batch

BF16 = ml_dtypes.bfloat16
_dt = mybir.dt


def _pack_idx(idx: np.ndarray) -> np.ndarray:
    """[n] -> [128, n/16] int16 (stripe-of-16 column-major, replicated x8)."""
    n = idx.shape[0]
    assert n % 16 == 0
    t16 = idx.astype(np.int16).reshape(n // 16, 16).T
    return np.tile(t16, (8, 1))


def _host_prep(h, src, dst):
    """Per-core edge layout. Returns static plan + per-core arrays."""
    core_of = dst // NPC
    percore = []
    for c in range(CORES):
        sel = np.nonzero(core_of == c)[0]
        e_src = src[sel]
        e_dst = dst[sel] - c * NPC
        order = np.argsort(e_dst, kind="stable")
        e_src = e_src[order]
        e_dst = e_dst[order]
        own_lo = c * NPC
        pos = np.empty(N_NODES, np.int64)
        own = np.arange(own_lo, own_lo + NPC)
        others = np.concatenate([np.arange(0, own_lo), np.arange(own_lo + NPC, N_NODES)])
        perm = np.concatenate([own, others])        # table row r holds node perm[r]
        pos[perm] = np.arange(N_NODES)
        src_p = pos[e_src]
        w = e_dst // WIN
        is_lo = src_p < SPLIT
        percore.append(dict(src_p=src_p, e_dst=e_dst, w=w, is_lo=is_lo, perm=perm))

    T_lo = np.zeros(NWIN, np.int64)
    T_hi = np.zeros(NWIN, np.int64)
    for c in range(CORES):
        pc = percore[c]
        for w in range(NWIN):
            m = pc["w"] == w
            nlo = int((m & pc["is_lo"]).sum())
            nhi = int((m & ~pc["is_lo"]).sum())
            T_lo[w] = max(T_lo[w], (nlo + 127) // 128)
            T_hi[w] = max(T_hi[w], (nhi + 127) // 128)
    T_lo = np.maximum(T_lo, 1)
    T_hi = np.maximum(T_hi, 1)

    TT = int((T_lo + T_hi).sum())
    LO_TOT = int(T_lo.sum()) * 128
    HI_TOT = int(T_hi.sum()) * 128

    arrs = []
    for c in range(CORES):
        pc = percore[c]
        ilo = np.zeros(LO_TOT, np.int64)
        ihi = np.zeros(HI_TOT, np.int64)
        oh = np.zeros((128, TT * 128), dtype=BF16)
        ohT = np.zeros((128, TT * 128), dtype=BF16)
        lo_off = 0
        hi_off = 0
        proc = 0
        for w in range(NWIN):
            m = pc["w"] == w
            for cls in (0, 1):
                if cls == 0:
                    sel = np.nonzero(m & pc["is_lo"])[0]
                    ntile = int(T_lo[w])
                    vals = pc["src_p"][sel]
                else:
                    sel = np.nonzero(m & ~pc["is_lo"])[0]
                    ntile = int(T_hi[w])
                    vals = pc["src_p"][sel] - SPLIT
                cnt = sel.shape[0]
                assert ntile * 128 - cnt >= 0
                if cls == 0:
                    ilo[lo_off:lo_off + cnt] = vals
                    lo_off += ntile * 128
                else:
                    ihi[hi_off:hi_off + cnt] = vals
                    hi_off += ntile * 128
                dstrel = pc["e_dst"][sel] - w * WIN
                slot = np.arange(cnt)
                tile_i = proc + slot // 128
                oh[slot % 128, tile_i * 128 + dstrel] = 1.0
                ohT[dstrel, tile_i * 128 + slot % 128] = 1.0
                proc += ntile
        assert proc == TT
        arrs.append(dict(
            ilo=_pack_idx(ilo), ihi=_pack_idx(ihi), oh=oh, ohT=ohT,
            perm=pc["perm"],
        ))
    return dict(T_lo=T_lo, T_hi=T_hi, TT=TT, LO_TOT=LO_TOT, HI_TOT=HI_TOT), arrs


def _build_program(plan):
    STAGE = os.environ.get("KSTAGE", "full")
    T_lo, T_hi, TT = plan["T_lo"], plan["T_hi"], plan["TT"]
    LO_TOT, HI_TOT = plan["LO_TOT"], plan["HI_TOT"]

    nc = bacc.Bacc("TRN2", target_bir_lowering=False, debug=False, num_swdge_queues=4)
    hT = nc.dram_tensor("hT", [128, N_NODES], _dt.bfloat16, kind="ExternalInput")
    wkvq = nc.dram_tensor("wkvq", [128, KVQ_W], _dt.bfloat16, kind="ExternalInput")
    bqb = nc.dram_tensor("bqb", [128, F], _dt.bfloat16, kind="ExternalInput")
    bkb = nc.dram_tensor("bkb", [128, F], _dt.bfloat16, kind="ExternalInput")
    bvb = nc.dram_tensor("bvb", [128, F], _dt.bfloat16, kind="ExternalInput")
    ilo = nc.dram_tensor("ilo", [128, LO_TOT // 16], _dt.int16, kind="ExternalInput")
    ihi = nc.dram_tensor("ihi", [128, HI_TOT // 16], _dt.int16, kind="ExternalInput")
    oh = nc.dram_tensor("oh", [128, TT * 128], _dt.bfloat16, kind="ExternalInput")
    ohT = nc.dram_tensor("ohT", [128, TT * 128], _dt.bfloat16, kind="ExternalInput")
    out = nc.dram_tensor("out", [Q_ROWS, F], _dt.float32, kind="ExternalOutput")

    sbs = []
    w0 = 0
    while w0 < NWIN:
        sbs.append(list(range(w0, min(w0 + SB_WINDOWS, NWIN))))
        w0 += SB_WINDOWS

    with tile.TileContext(nc) as tc:
        with (
            tc.tile_pool(name="const", bufs=1) as constp,
            tc.tile_pool(name="dram", bufs=1, space="DRAM") as dramp,
            tc.tile_pool(name="prep_in", bufs=3) as prep_in,
            tc.tile_pool(name="prep_ps", bufs=2, space="PSUM") as prep_ps,
            tc.tile_pool(name="prep_out", bufs=3) as prep_out,
            tc.tile_pool(name="gath", bufs=2) as gath,
            tc.tile_pool(name="work", bufs=4) as work,
            tc.tile_pool(name="mps", bufs=2, space="PSUM") as mps,
            tc.tile_pool(name="fin", bufs=3) as finp,
        ):
            nc.gpsimd.load_library(library_config.mlp)

            wk_t = constp.tile([128, KVQ_W], _dt.bfloat16)
            nc.sync.dma_start(wk_t[:], wkvq[:, :])
            bq_t = constp.tile([128, F], _dt.bfloat16)
            nc.sync.dma_start(bq_t[:], bqb[:, :])
            bk_t = constp.tile([128, F], _dt.bfloat16)
            nc.sync.dma_start(bk_t[:], bkb[:, :])
            bv_t = constp.tile([128, F], _dt.bfloat16)
            nc.sync.dma_start(bv_t[:], bvb[:, :])

            kvq = dramp.tile([PREP_TILES * 128, KVQ_W], _dt.bfloat16)

            # ---------------- prep: KVQ table ----------------
            g0 = 0
            while g0 < PREP_TILES:
                g1 = min(g0 + PREP_GROUP, PREP_TILES)
                ng = g1 - g0
                r0 = g0 * 128
                ncol = min(ng * 128, N_NODES - r0) if r0 < N_NODES else 0
                htg = prep_in.tile([128, PREP_GROUP * 128], _dt.bfloat16, tag="htg")
                nc.sync.dma_start(htg[:, :ncol], hT[:, r0:r0 + ncol])
                kvg = prep_out.tile([128, PREP_GROUP, KVQ_W], _dt.bfloat16, tag="kvg")
                for j0 in range(0, ng, 2):
                    j1 = min(j0 + 2, ng)
                    # [128, 2, 512]: each 512-fp32 slot is one PSUM bank
                    ps = prep_ps.tile([128, 2, 512], _dt.float32, space="PSUM", tag="pps")
                    for j in range(j0, j1):
                        nc.tensor.matmul(
                            ps[:, j - j0, 0:KVQ_W],
                            lhsT=htg[:, j * 128:(j + 1) * 128], rhs=wk_t[:],
                            start=True, stop=True,
                        )
                    nc.scalar.copy(kvg[:, j0:j1, :], ps[:, 0:j1 - j0, 0:KVQ_W])
                nc.scalar.dma_start(
                    kvq[r0:r0 + ng * 128, :].rearrange("(t p) f -> p t f", p=128),
                    kvg[:, :ng, :],
                )
                g0 = g1

            # ---------------- main ----------------
            if STAGE == "prep":
                probe = finp.tile([128, F], _dt.float32, tag="probe")
                ksl = finp.tile([128, F], _dt.bfloat16, tag="ksl")
                nc.sync.dma_start(ksl[:], kvq[0:128, 0:F])
                nc.vector.tensor_copy(probe[:], ksl[:])
                nc.sync.dma_start(out[0:128, :], probe[:])
                sbs = []

            lo_pos = 0
            hi_pos = 0
            proc = 0
            for sb in sbs:
                nlo = int(sum(T_lo[w] for w in sb))
                nhi = int(sum(T_hi[w] for w in sb))
                nt = nlo + nhi
                ilo_t = gath.tile([128, nlo * 8], _dt.int16, tag="ilo")
                nc.sync.dma_start(ilo_t[:], ilo[:, lo_pos * 8:(lo_pos + nlo) * 8])
                ihi_t = gath.tile([128, nhi * 8], _dt.int16, tag="ihi")
                nc.sync.dma_start(ihi_t[:], ihi[:, hi_pos * 8:(hi_pos + nhi) * 8])
                oh_t = gath.tile([128, nt * 128], _dt.bfloat16, tag="oh")
                nc.scalar.dma_start(oh_t[:], oh[:, proc * 128:(proc + nt) * 128])
                ohT_t = gath.tile([128, nt * 128], _dt.bfloat16, tag="ohT")
                nc.scalar.dma_start(ohT_t[:], ohT[:, proc * 128:(proc + nt) * 128])

                kvlo = gath.tile([128, nlo, 2 * F], _dt.bfloat16, tag="kvlo")
                kvhi = gath.tile([128, nhi, 2 * F], _dt.bfloat16, tag="kvhi")
                # queue-striped gathers: 4 SWDGE queues run on distinct Q7
                # pairs and overlap their descriptor generation.
                for buf, idxt, n_t, base in (
                    (kvlo, ilo_t, nlo, 0),
                    (kvhi, ihi_t, nhi, SPLIT),
                ):
                    table = kvq[base:SPLIT if base == 0 else N_NODES, 0:2 * F]
                    step = (n_t + 3) // 4
                    t0 = 0
                    q = 0
                    while t0 < n_t:
                        t1 = min(t0 + step, n_t)
                        nsub = t1 - t0
                        nc.gpsimd.dma_gather(
                            buf[:, t0:t1, :], table, idxt[:, t0 * 8:t1 * 8],
                            nsub * 128, nsub * 128, 2 * F,
                            elem_step=KVQ_W, single_packet=False, queue_num=q,
                        )
                        t0 = t1
                        q = (q + 1) % 4

                sb_lo = 0
                sb_hi = 0
                sb_proc = 0
                for w in sb:
                    qwt = work.tile([128, F + H], _dt.bfloat16, tag="qw")
                    nc.scalar.dma_start(qwt[:, 0:F], kvq[w * WIN:(w + 1) * WIN, 2 * F:3 * F])
                    nc.gpsimd.tensor_tensor(
                        out=qwt[:, 0:F], in0=qwt[:, 0:F], in1=bq_t[:],
                        op=mybir.AluOpType.add,
                    )
                    t1p = finp.tile([128, F], _dt.bfloat16, tag="t1p")
                    nc.gpsimd.tensor_tensor(
                        out=t1p[:], in0=qwt[:, 0:F], in1=bk_t[:],
                        op=mybir.AluOpType.mult,
                    )
                    t1f = finp.tile([128, H], _dt.float32, tag="t1f")
                    nc.vector.tensor_reduce(
                        out=t1f[:],
                        in_=t1p[:].rearrange("p (h d) -> p h d", h=H, d=D),
                        axis=mybir.AxisListType.X,
                        op=mybir.AluOpType.add,
                    )
                    nc.scalar.copy(qwt[:, F:F + H], t1f[:])
                    outz_ps = mps.tile([128, F + H], _dt.float32, space="PSUM", tag="outz")
                    wtiles = int(T_lo[w] + T_hi[w])
                    kdone = 0
                    for cls in (0, 1):
                        tc_n = int(T_lo[w]) if cls == 0 else int(T_hi[w])
                        if tc_n == 0:
                            continue
                        if cls == 0:
                            kv_g, kpos = kvlo, sb_lo
                            sb_lo += tc_n
                        else:
                            kv_g, kpos = kvhi, sb_hi
                            sb_hi += tc_n
                        K_ap = kv_g[:, kpos:kpos + tc_n, 0:F]
                        V_ap = kv_g[:, kpos:kpos + tc_n, F:2 * F]

                        # Q expansion: per tile one-hot^T @ Q_win; 4 tiles
                        # share one PSUM bank and one batched evacuation.
                        qe = work.tile([128, tc_n, F + H], _dt.bfloat16, tag="qe")
                        k0 = 0
                        while k0 < tc_n:
                            k1 = min(k0 + 3, tc_n)
                            qe_ps = mps.tile([128, 3, F + H], _dt.float32, space="PSUM", tag="qeps")
                            for k in range(k0, k1):
                                nc.tensor.matmul(
                                    qe_ps[:, k - k0, :],
                                    lhsT=ohT_t[:, (sb_proc + k) * 128:(sb_proc + k + 1) * 128],
                                    rhs=qwt[:],
                                    start=True, stop=True,
                                )
                            nc.scalar.copy(qe[:, k0:k1, :], qe_ps[:, 0:k1 - k0, :])
                            k0 = k1

                        kq = work.tile([128, tc_n, F], _dt.bfloat16, tag="kq")
                        nc.vector.tensor_tensor(
                            out=kq[:], in0=K_ap, in1=qe[:, :, 0:F], op=mybir.AluOpType.mult
                        )
                        raw = work.tile([128, tc_n * H], _dt.float32, tag="raw")
                        nc.vector.tensor_reduce(
                            out=raw[:],
                            in_=kq[:].rearrange("p t (h d) -> p (t h) d", h=H, d=D),
                            axis=mybir.AxisListType.X,
                            op=mybir.AluOpType.add,
                        )
                        nc.vector.tensor_tensor(
                            out=raw[:].rearrange("p (t h) -> p t h", h=H),
                            in0=raw[:].rearrange("p (t h) -> p t h", h=H),
                            in1=qe[:, :, F:F + H],
                            op=mybir.AluOpType.add,
                        )
                        nc.vector.tensor_scalar(
                            out=raw[:], in0=raw[:],
                            scalar1=20.0, scalar2=-20.0,
                            op0=mybir.AluOpType.min, op1=mybir.AluOpType.max,
                        )
                        wv_s = work.tile([128, tc_n, F + H], _dt.bfloat16, tag="wvs")
                        nc.scalar.activation(
                            wv_s[:, :, F:F + H],
                            raw[:].rearrange("p (t h) -> p t h", h=H),
                            mybir.ActivationFunctionType.Exp, scale=0.25,
                        )
                        sbc = work.tile([128, tc_n, F], _dt.bfloat16, tag="sbc")
                        s_base = wv_s[:, 0:tc_n, F:F + H]
                        s_b = bass.AP(
                            s_base.tensor, s_base.offset,
                            [s_base.ap[0], [F + H, tc_n], [1, H], [0, D]],
                        )
                        nc.scalar.activation(
                            sbc[:].rearrange("p t (g d) -> p t g d", d=D),
                            s_b,
                            mybir.ActivationFunctionType.Copy,
                        )
                        nc.vector.tensor_tensor(
                            out=wv_s[:, :, 0:F],
                            in0=V_ap,
                            in1=sbc[:],
                            op=mybir.AluOpType.mult,
                        )
                        for k in range(tc_n):
                            nc.tensor.matmul(
                                outz_ps[:],
                                lhsT=oh_t[:, (sb_proc + k) * 128:(sb_proc + k + 1) * 128],
                                rhs=wv_s[:, k, :],
                                start=(kdone == 0), stop=(kdone == wtiles - 1),
                            )
                            kdone += 1
                        sb_proc += tc_n
                    # finalize window
                    zf = finp.tile([128, H], _dt.float32, tag="zf")
                    nc.vector.tensor_scalar_add(
                        out=zf[:], in0=outz_ps[:, F:F + H], scalar1=1e-6
                    )
                    rz = finp.tile([128, H], _dt.float32, tag="rz")
                    nc.vector.reciprocal(rz[:], zf[:])
                    fin = finp.tile([128, F], _dt.float32, tag="fin")
                    zb = bass.AP(zf.tensor, zf[:].offset, [zf[:].ap[0], [1, H], [0, D]])
                    term = finp.tile([128, F], _dt.float32, tag="term")
                    nc.vector.tensor_tensor(
                        out=term[:].rearrange("p (h d) -> p h d", h=H, d=D),
                        in0=bv_t[:].rearrange("p (h d) -> p h d", h=H, d=D),
                        in1=zb,
                        op=mybir.AluOpType.mult,
                    )
                    nc.vector.tensor_tensor(
                        out=term[:], in0=term[:], in1=outz_ps[:, 0:F],
                        op=mybir.AluOpType.add,
                    )
                    rz_b = bass.AP(rz.tensor, rz[:].offset, [rz[:].ap[0], [1, H], [0, D]])
                    nc.vector.tensor_tensor(
                        out=fin[:].rearrange("p (h d) -> p h d", h=H, d=D),
                        in0=term[:].rearrange("p (h d) -> p h d", h=H, d=D),
                        in1=rz_b,
                        op=mybir.AluOpType.mult,
                    )
                    nvalid = min(WIN, NPC - w * WIN)
                    nc.sync.dma_start(out[w * WIN:w * WIN + nvalid, :], fin[:nvalid, :])
                lo_pos += nlo
                hi_pos += nhi
                proc += nt

    nc.compile()
    return nc


def kernel(**inputs):
    h = np.asarray(inputs["h"], np.float32)
    src = np.asarray(inputs["src"]).astype(np.int64)
    dst = np.asarray(inputs["dst"]).astype(np.int64)
    Wq = np.asarray(inputs["Wq"], np.float32)
    bq = np.asarray(inputs["bq"], np.float32)
    Wk = np.asarray(inputs["Wk"], np.float32)
    bk = np.asarray(inputs["bk"], np.float32)
    Wv = np.asarray(inputs["Wv"], np.float32)
    bv = np.asarray(inputs["bv"], np.float32)

    plan, arrs = _host_prep(h, src, dst)
    nc = _build_program(plan)

    wkvq = np.concatenate([Wk, Wv, Wq], axis=1).astype(BF16)
    bqb = np.ascontiguousarray(np.broadcast_to(bq, (128, F))).astype(BF16)
    bkb = np.ascontiguousarray(np.broadcast_to(bk, (128, F))).astype(BF16)
    bvb = np.ascontiguousarray(np.broadcast_to(bv, (128, F))).astype(BF16)

    in_maps = []
    for c in range(CORES):
        a = arrs[c]
        hTc = np.ascontiguousarray(h[a["perm"], :].T).astype(BF16)
        in_maps.append({
            "hT": hTc,
            "wkvq": wkvq,
            "bqb": bqb,
            "bkb": bkb,
            "bvb": bvb,
            "ilo": a["ilo"],
            "ihi": a["ihi"],
            "oh": a["oh"],
            "ohT": a["ohT"],
        })

    res = run_bass_kernel_spmd(nc, in_maps, core_ids=list(range(CORES)))
    out = np.concatenate(
        [res.results[c]["out"][:NPC] for c in range(CORES)], axis=0
    )
    return out.reshape(N_NODES, H, D)


# revision 19
# speedup vs baseline: 1.3011x; 1.3011x over previous
"""Graph attention (BatchedAttentionLayer) Bass kernel for 8 trn2 NeuronCores.

Full-input contract: kernel(**inputs) -> [50000, 8, 16] float32.

Strategy (sharded by destination node):
  - 8 cores x 6250 dst nodes; edges routed to the core owning their dst,
    sorted by dst into 49 windows of 128 dst slots, tiled in 128-edge tiles.
  - Per-core node permutation puts own nodes first so the SPMD program is
    identical across cores.
  - Device prep (bf16): KVQ table [50048, 384] = [h@Wk+bk | h@Wv+bv | h@Wq+bq]
    via TensorE (h_T chunks stationary, bias via ones-row matmul into the
    same PSUM accumulation group).
  - Main: dma_gather pulls K|V rows (512B bf16, int16 idx => table split at
    row 32768); Q comes from per-window table loads expanded per tile with a
    one-hot matmul (no per-edge Q gather).  DVE: K*Q, segmented head-reduce,
    fused clip; ACT: exp(0.25*raw), s head-broadcast; DVE: V*s; TensorE: one
    fused scatter matmul per tile (rhs = [wV | s]) accumulating out+z in one
    PSUM bank per window; reciprocal-divide; DMA out.
"""

import os

import numpy as np
import ml_dtypes

import concourse.bacc as bacc
import concourse.bass as bass
import concourse.mybir as mybir
import concourse.tile as tile
from concourse import library_config
from concourse.bass_utils import run_bass_kernel_spmd

N_NODES = 50000
N_EDGES = 800000
F = 128            # feature dim = H*D
H = 8
D = 16
CORES = 8
NPC = N_NODES // CORES           # 6250 nodes per core
WIN = 128                        # dst nodes per window
NWIN = (NPC + WIN - 1) // WIN    # 49 windows per core
SPLIT = 32768                    # int16-safe KV table split row
KVQ_W = F * 3                    # 384: K | V | Q columns
PREP_TILES = (N_NODES + 127) // 128   # 391
PREP_GROUP = 4                   # node tiles per prep DMA group
Q_ROWS = NWIN * WIN              # 6272
SB_WINDOWS = 3                   # windows per gather super-batch

BF16 = ml_dtypes.bfloat16
_dt = mybir.dt


def _pack_idx(idx: np.ndarray) -> np.ndarray:
    """[n] -> [128, n/16] int16 (stripe-of-16 column-major, replicated x8)."""
    n = idx.shape[0]
    assert n % 16 == 0
    t16 = idx.astype(np.int16).reshape(n // 16, 16).T
    return np.tile(t16, (8, 1))


def _host_prep(h, src, dst):
    """Per-core edge layout. Returns static plan + per-core arrays."""
    core_of = dst // NPC
    percore = []
    for c in range(CORES):
        sel = np.nonzero(core_of == c)[0]
        e_src = src[sel]
        e_dst = dst[sel] - c * NPC
        order = np.argsort(e_dst, kind="stable")
        e_src = e_src[order]
        e_dst = e_dst[order]
        own_lo = c * NPC
        pos = np.empty(N_NODES, np.int64)
        own = np.arange(own_lo, own_lo + NPC)
        others = np.concatenate([np.arange(0, own_lo), np.arange(own_lo + NPC, N_NODES)])
        perm = np.concatenate([own, others])        # table row r holds node perm[r]
        pos[perm] = np.arange(N_NODES)
        src_p = pos[e_src]
        w = e_dst // WIN
        is_lo = src_p < SPLIT
        percore.append(dict(src_p=src_p, e_dst=e_dst, w=w, is_lo=is_lo, perm=perm))

    T_lo = np.zeros(NWIN, np.int64)
    T_hi = np.zeros(NWIN, np.int64)
    for c in range(CORES):
        pc = percore[c]
        for w in range(NWIN):
            m = pc["w"] == w
            nlo = int((m & pc["is_lo"]).sum())
            nhi = int((m & ~pc["is_lo"]).sum())
            T_lo[w] = max(T_lo[w], (nlo + 127) // 128)
            T_hi[w] = max(T_hi[w], (nhi + 127) // 128)
    T_lo = np.maximum(T_lo, 1)
    T_hi = np.maximum(T_hi, 1)

    TT = int((T_lo + T_hi).sum())
    LO_TOT = int(T_lo.sum()) * 128
    HI_TOT = int(T_hi.sum()) * 128

    arrs = []
    for c in range(CORES):
        pc = percore[c]
        ilo = np.zeros(LO_TOT, np.int64)
        ihi = np.zeros(HI_TOT, np.int64)
        oh = np.zeros((128, TT * 128), dtype=BF16)
        ohT = np.zeros((128, TT * 128), dtype=BF16)
        lo_off = 0
        hi_off = 0
        proc = 0
        for w in range(NWIN):
            m = pc["w"] == w
            for cls in (0, 1):
                if cls == 0:
                    sel = np.nonzero(m & pc["is_lo"])[0]
                    ntile = int(T_lo[w])
                    vals = pc["src_p"][sel]
                else:
                    sel = np.nonzero(m & ~pc["is_lo"])[0]
                    ntile = int(T_hi[w])
                    vals = pc["src_p"][sel] - SPLIT
                cnt = sel.shape[0]
                assert ntile * 128 - cnt >= 0
                if cls == 0:
                    ilo[lo_off:lo_off + cnt] = vals
                    lo_off += ntile * 128
                else:
                    ihi[hi_off:hi_off + cnt] = vals
                    hi_off += ntile * 128
                dstrel = pc["e_dst"][sel] - w * WIN
                slot = np.arange(cnt)
                tile_i = proc + slot // 128
                oh[slot % 128, tile_i * 128 + dstrel] = 1.0
                ohT[dstrel, tile_i * 128 + slot % 128] = 1.0
                proc += ntile
        assert proc == TT
        arrs.append(dict(
            ilo=_pack_idx(ilo), ihi=_pack_idx(ihi), oh=oh, ohT=ohT,
            perm=pc["perm"],
        ))
    return dict(T_lo=T_lo, T_hi=T_hi, TT=TT, LO_TOT=LO_TOT, HI_TOT=HI_TOT), arrs


def _build_program(plan):
    STAGE = os.environ.get("KSTAGE", "full")
    T_lo, T_hi, TT = plan["T_lo"], plan["T_hi"], plan["TT"]
    LO_TOT, HI_TOT = plan["LO_TOT"], plan["HI_TOT"]

    nc = bacc.Bacc("TRN2", target_bir_lowering=False, debug=False, num_swdge_queues=4)
    hT = nc.dram_tensor("hT", [128, N_NODES], _dt.bfloat16, kind="ExternalInput")
    wkvq = nc.dram_tensor("wkvq", [128, KVQ_W], _dt.bfloat16, kind="ExternalInput")
    bqb = nc.dram_tensor("bqb", [128, F], _dt.bfloat16, kind="ExternalInput")
    bkb = nc.dram_tensor("bkb", [128, F], _dt.bfloat16, kind="ExternalInput")
    bvb = nc.dram_tensor("bvb", [128, F], _dt.bfloat16, kind="ExternalInput")
    ilo = nc.dram_tensor("ilo", [128, LO_TOT // 16], _dt.int16, kind="ExternalInput")
    ihi = nc.dram_tensor("ihi", [128, HI_TOT // 16], _dt.int16, kind="ExternalInput")
    oh = nc.dram_tensor("oh", [128, TT * 128], _dt.bfloat16, kind="ExternalInput")
    ohT = nc.dram_tensor("ohT", [128, TT * 128], _dt.bfloat16, kind="ExternalInput")
    out = nc.dram_tensor("out", [Q_ROWS, F], _dt.float32, kind="ExternalOutput")

    sbs = []
    w0 = 0
    while w0 < NWIN:
        sbs.append(list(range(w0, min(w0 + SB_WINDOWS, NWIN))))
        w0 += SB_WINDOWS

    with tile.TileContext(nc) as tc:
        with (
            tc.tile_pool(name="const", bufs=1) as constp,
            tc.tile_pool(name="dram", bufs=1, space="DRAM") as dramp,
            tc.tile_pool(name="prep_in", bufs=3) as prep_in,
            tc.tile_pool(name="prep_ps", bufs=2, space="PSUM") as prep_ps,
            tc.tile_pool(name="prep_out", bufs=3) as prep_out,
            tc.tile_pool(name="gath", bufs=2) as gath,
            tc.tile_pool(name="work", bufs=4) as work,
            tc.tile_pool(name="mps", bufs=2, space="PSUM") as mps,
            tc.tile_pool(name="fin", bufs=3) as finp,
        ):
            nc.gpsimd.load_library(library_config.mlp)

            wk_t = constp.tile([128, KVQ_W], _dt.bfloat16)
            nc.sync.dma_start(wk_t[:], wkvq[:, :])
            bq_t = constp.tile([128, F], _dt.bfloat16)
            nc.sync.dma_start(bq_t[:], bqb[:, :])
            bk_t = constp.tile([128, F], _dt.bfloat16)
            nc.sync.dma_start(bk_t[:], bkb[:, :])
            bv_t = constp.tile([128, F], _dt.bfloat16)
            nc.sync.dma_start(bv_t[:], bvb[:, :])

            kvq = dramp.tile([PREP_TILES * 128, KVQ_W], _dt.bfloat16)

            # ---------------- prep: KVQ table ----------------
            g0 = 0
            while g0 < PREP_TILES:
                g1 = min(g0 + PREP_GROUP, PREP_TILES)
                ng = g1 - g0
                r0 = g0 * 128
                ncol = min(ng * 128, N_NODES - r0) if r0 < N_NODES else 0
                htg = prep_in.tile([128, PREP_GROUP * 128], _dt.bfloat16, tag="htg")
                nc.sync.dma_start(htg[:, :ncol], hT[:, r0:r0 + ncol])
                kvg = prep_out.tile([128, PREP_GROUP, KVQ_W], _dt.bfloat16, tag="kvg")
                for j0 in range(0, ng, 2):
                    j1 = min(j0 + 2, ng)
                    # [128, 2, 512]: each 512-fp32 slot is one PSUM bank
                    ps = prep_ps.tile([128, 2, 512], _dt.float32, space="PSUM", tag="pps")
                    for j in range(j0, j1):
                        nc.tensor.matmul(
                            ps[:, j - j0, 0:KVQ_W],
                            lhsT=htg[:, j * 128:(j + 1) * 128], rhs=wk_t[:],
                            start=True, stop=True,
                        )
                    nc.scalar.copy(kvg[:, j0:j1, :], ps[:, 0:j1 - j0, 0:KVQ_W])
                nc.scalar.dma_start(
                    kvq[r0:r0 + ng * 128, :].rearrange("(t p) f -> p t f", p=128),
                    kvg[:, :ng, :],
                )
                g0 = g1

            # ---------------- main ----------------
            if STAGE == "prep":
                probe = finp.tile([128, F], _dt.float32, tag="probe")
                ksl = finp.tile([128, F], _dt.bfloat16, tag="ksl")
                nc.sync.dma_start(ksl[:], kvq[0:128, 0:F])
                nc.vector.tensor_copy(probe[:], ksl[:])
                nc.sync.dma_start(out[0:128, :], probe[:])
                sbs = []

            lo_pos = 0
            hi_pos = 0
            proc = 0
            for sb in sbs:
                nlo = int(sum(T_lo[w] for w in sb))
                nhi = int(sum(T_hi[w] for w in sb))
                nt = nlo + nhi
                ilo_t = gath.tile([128, nlo * 8], _dt.int16, tag="ilo")
                nc.sync.dma_start(ilo_t[:], ilo[:, lo_pos * 8:(lo_pos + nlo) * 8])
                ihi_t = gath.tile([128, nhi * 8], _dt.int16, tag="ihi")
                nc.sync.dma_start(ihi_t[:], ihi[:, hi_pos * 8:(hi_pos + nhi) * 8])
                oh_t = gath.tile([128, nt * 128], _dt.bfloat16, tag="oh")
                nc.scalar.dma_start(oh_t[:], oh[:, proc * 128:(proc + nt) * 128])
                ohT_t = gath.tile([128, nt * 128], _dt.bfloat16, tag="ohT")
                nc.scalar.dma_start(ohT_t[:], ohT[:, proc * 128:(proc + nt) * 128])

                kvlo = gath.tile([128, nlo, 2 * F], _dt.bfloat16, tag="kvlo")
                kvhi = gath.tile([128, nhi, 2 * F], _dt.bfloat16, tag="kvhi")
                # queue-striped gathers: 4 SWDGE queues run on distinct Q7
                # pairs and overlap their descriptor generation.
                for buf, idxt, n_t, base in (
                    (kvlo, ilo_t, nlo, 0),
                    (kvhi, ihi_t, nhi, SPLIT),
                ):
                    table = kvq[base:SPLIT if base == 0 else N_NODES, 0:2 * F]
                    step = (n_t + 3) // 4
                    t0 = 0
                    q = 0
                    while t0 < n_t:
                        t1 = min(t0 + step, n_t)
                        nsub = t1 - t0
                        nc.gpsimd.dma_gather(
                            buf[:, t0:t1, :], table, idxt[:, t0 * 8:t1 * 8],
                            nsub * 128, nsub * 128, 2 * F,
                            elem_step=KVQ_W, single_packet=False, queue_num=q,
                        )
                        t0 = t1
                        q = (q + 1) % 4

                sb_lo = 0
                sb_hi = 0
                sb_proc = 0
                for w in sb:
                    qwt = work.tile([128, F + H], _dt.bfloat16, tag="qw")
                    nc.scalar.dma_start(qwt[:, 0:F], kvq[w * WIN:(w + 1) * WIN, 2 * F:3 * F])
                    nc.vector.tensor_tensor(
                        out=qwt[:, 0:F], in0=qwt[:, 0:F], in1=bq_t[:],
                        op=mybir.AluOpType.add,
                    )
                    t1p = finp.tile([128, F], _dt.bfloat16, tag="t1p")
                    nc.vector.tensor_tensor(
                        out=t1p[:], in0=qwt[:, 0:F], in1=bk_t[:],
                        op=mybir.AluOpType.mult,
                    )
                    t1f = finp.tile([128, H], _dt.float32, tag="t1f")
                    nc.vector.tensor_reduce(
                        out=t1f[:],
                        in_=t1p[:].rearrange("p (h d) -> p h d", h=H, d=D),
                        axis=mybir.AxisListType.X,
                        op=mybir.AluOpType.add,
                    )
                    nc.scalar.copy(qwt[:, F:F + H], t1f[:])
                    outz_ps = mps.tile([128, F + H], _dt.float32, space="PSUM", tag="outz")
                    wtiles = int(T_lo[w] + T_hi[w])
                    kdone = 0
                    for cls in (0, 1):
                        tc_n = int(T_lo[w]) if cls == 0 else int(T_hi[w])
                        if tc_n == 0:
                            continue
                        if cls == 0:
                            kv_g, kpos = kvlo, sb_lo
                            sb_lo += tc_n
                        else:
                            kv_g, kpos = kvhi, sb_hi
                            sb_hi += tc_n
                        K_ap = kv_g[:, kpos:kpos + tc_n, 0:F]
                        V_ap = kv_g[:, kpos:kpos + tc_n, F:2 * F]

                        # Q expansion: per tile one-hot^T @ Q_win; 4 tiles
                        # share one PSUM bank and one batched evacuation.
                        qe = work.tile([128, tc_n, F + H], _dt.bfloat16, tag="qe")
                        k0 = 0
                        while k0 < tc_n:
                            k1 = min(k0 + 3, tc_n)
                            qe_ps = mps.tile([128, 3, F + H], _dt.float32, space="PSUM", tag="qeps")
                            for k in range(k0, k1):
                                nc.tensor.matmul(
                                    qe_ps[:, k - k0, :],
                                    lhsT=ohT_t[:, (sb_proc + k) * 128:(sb_proc + k + 1) * 128],
                                    rhs=qwt[:],
                                    start=True, stop=True,
                                )
                            nc.scalar.copy(qe[:, k0:k1, :], qe_ps[:, 0:k1 - k0, :])
                            k0 = k1

                        kq = work.tile([128, tc_n, F], _dt.bfloat16, tag="kq")
                        nc.vector.tensor_tensor(
                            out=kq[:], in0=K_ap, in1=qe[:, :, 0:F], op=mybir.AluOpType.mult
                        )
                        raw = work.tile([128, tc_n * H], _dt.float32, tag="raw")
                        nc.vector.tensor_reduce(
                            out=raw[:],
                            in_=kq[:].rearrange("p t (h d) -> p (t h) d", h=H, d=D),
                            axis=mybir.AxisListType.X,
                            op=mybir.AluOpType.add,
                        )
                        nc.vector.tensor_tensor(
                            out=raw[:].rearrange("p (t h) -> p t h", h=H),
                            in0=raw[:].rearrange("p (t h) -> p t h", h=H),
                            in1=qe[:, :, F:F + H],
                            op=mybir.AluOpType.add,
                        )
                        nc.vector.tensor_scalar(
                            out=raw[:], in0=raw[:],
                            scalar1=20.0, scalar2=-20.0,
                            op0=mybir.AluOpType.min, op1=mybir.AluOpType.max,
                        )
                        wv_s = work.tile([128, tc_n, F + H], _dt.bfloat16, tag="wvs")
                        nc.scalar.activation(
                            wv_s[:, :, F:F + H],
                            raw[:].rearrange("p (t h) -> p t h", h=H),
                            mybir.ActivationFunctionType.Exp, scale=0.25,
                        )
                        sbc = work.tile([128, tc_n, F], _dt.bfloat16, tag="sbc")
                        s_base = wv_s[:, 0:tc_n, F:F + H]
                        s_b = bass.AP(
                            s_base.tensor, s_base.offset,
                            [s_base.ap[0], [F + H, tc_n], [1, H], [0, D]],
                        )
                        nc.scalar.activation(
                            sbc[:].rearrange("p t (g d) -> p t g d", d=D),
                            s_b,
                            mybir.ActivationFunctionType.Copy,
                        )
                        nc.vector.tensor_tensor(
                            out=wv_s[:, :, 0:F],
                            in0=V_ap,
                            in1=sbc[:],
                            op=mybir.AluOpType.mult,
                        )
                        for k in range(tc_n):
                            nc.tensor.matmul(
                                outz_ps[:],
                                lhsT=oh_t[:, (sb_proc + k) * 128:(sb_proc + k + 1) * 128],
                                rhs=wv_s[:, k, :],
                                start=(kdone == 0), stop=(kdone == wtiles - 1),
                            )
                            kdone += 1
                        sb_proc += tc_n
                    # finalize window
                    zf = finp.tile([128, H], _dt.float32, tag="zf")
                    nc.vector.tensor_scalar_add(
                        out=zf[:], in0=outz_ps[:, F:F + H], scalar1=1e-6
                    )
                    rz = finp.tile([128, H], _dt.float32, tag="rz")
                    nc.vector.reciprocal(rz[:], zf[:])
                    fin = finp.tile([128, F], _dt.float32, tag="fin")
                    zb = bass.AP(zf.tensor, zf[:].offset, [zf[:].ap[0], [1, H], [0, D]])
                    term = finp.tile([128, F], _dt.float32, tag="term")
                    nc.vector.tensor_tensor(
                        out=term[:].rearrange("p (h d) -> p h d", h=H, d=D),
                        in0=bv_t[:].rearrange("p (h d) -> p h d", h=H, d=D),
                        in1=zb,
                        op=mybir.AluOpType.mult,
                    )
                    nc.vector.tensor_tensor(
                        out=term[:], in0=term[:], in1=outz_ps[:, 0:F],
                        op=mybir.AluOpType.add,
                    )
                    rz_b = bass.AP(rz.tensor, rz[:].offset, [rz[:].ap[0], [1, H], [0, D]])
                    nc.vector.tensor_tensor(
                        out=fin[:].rearrange("p (h d) -> p h d", h=H, d=D),
                        in0=term[:].rearrange("p (h d) -> p h d", h=H, d=D),
                        in1=rz_b,
                        op=mybir.AluOpType.mult,
                    )
                    nvalid = min(WIN, NPC - w * WIN)
                    nc.sync.dma_start(out[w * WIN:w * WIN + nvalid, :], fin[:nvalid, :])
                lo_pos += nlo
                hi_pos += nhi
                proc += nt

    nc.compile()
    return nc


def kernel(**inputs):
    h = np.asarray(inputs["h"], np.float32)
    src = np.asarray(inputs["src"]).astype(np.int64)
    dst = np.asarray(inputs["dst"]).astype(np.int64)
    Wq = np.asarray(inputs["Wq"], np.float32)
    bq = np.asarray(inputs["bq"], np.float32)
    Wk = np.asarray(inputs["Wk"], np.float32)
    bk = np.asarray(inputs["bk"], np.float32)
    Wv = np.asarray(inputs["Wv"], np.float32)
    bv = np.asarray(inputs["bv"], np.float32)

    plan, arrs = _host_prep(h, src, dst)
    nc = _build_program(plan)

    wkvq = np.concatenate([Wk, Wv, Wq], axis=1).astype(BF16)
    bqb = np.ascontiguousarray(np.broadcast_to(bq, (128, F))).astype(BF16)
    bkb = np.ascontiguousarray(np.broadcast_to(bk, (128, F))).astype(BF16)
    bvb = np.ascontiguousarray(np.broadcast_to(bv, (128, F))).astype(BF16)

    in_maps = []
    for c in range(CORES):
        a = arrs[c]
        hTc = np.ascontiguousarray(h[a["perm"], :].T).astype(BF16)
        in_maps.append({
            "hT": hTc,
            "wkvq": wkvq,
            "bqb": bqb,
            "bkb": bkb,
            "bvb": bvb,
            "ilo": a["ilo"],
            "ihi": a["ihi"],
            "oh": a["oh"],
            "ohT": a["ohT"],
        })

    res = run_bass_kernel_spmd(nc, in_maps, core_ids=list(range(CORES)))
    out = np.concatenate(
        [res.results[c]["out"][:NPC] for c in range(CORES)], axis=0
    )
    return out.reshape(N_NODES, H, D)


# revision 20
# speedup vs baseline: 1.3091x; 1.0061x over previous
"""Graph attention (BatchedAttentionLayer) Bass kernel for 8 trn2 NeuronCores.

Full-input contract: kernel(**inputs) -> [50000, 8, 16] float32.

Strategy (sharded by destination node):
  - 8 cores x 6250 dst nodes; edges routed to the core owning their dst,
    sorted by dst into 49 windows of 128 dst slots, tiled in 128-edge tiles.
  - Per-core node permutation puts own nodes first so the SPMD program is
    identical across cores.
  - Device prep (bf16): KVQ table [50048, 384] = [h@Wk+bk | h@Wv+bv | h@Wq+bq]
    via TensorE (h_T chunks stationary, bias via ones-row matmul into the
    same PSUM accumulation group).
  - Main: dma_gather pulls K|V rows (512B bf16, int16 idx => table split at
    row 32768); Q comes from per-window table loads expanded per tile with a
    one-hot matmul (no per-edge Q gather).  DVE: K*Q, segmented head-reduce,
    fused clip; ACT: exp(0.25*raw), s head-broadcast; DVE: V*s; TensorE: one
    fused scatter matmul per tile (rhs = [wV | s]) accumulating out+z in one
    PSUM bank per window; reciprocal-divide; DMA out.
"""

import os

import numpy as np
import ml_dtypes

import concourse.bacc as bacc
import concourse.bass as bass
import concourse.mybir as mybir
import concourse.tile as tile
from concourse import library_config
from concourse.bass_utils import run_bass_kernel_spmd

N_NODES = 50000
N_EDGES = 800000
F = 128            # feature dim = H*D
H = 8
D = 16
CORES = 8
NPC = N_NODES // CORES           # 6250 nodes per core
WIN = 128                        # dst nodes per window
NWIN = (NPC + WIN - 1) // WIN    # 49 windows per core
SPLIT = 32768                    # int16-safe KV table split row
KVQ_W = F * 3                    # 384: K | V | Q columns
PREP_TILES = (N_NODES + 127) // 128   # 391
PREP_GROUP = 4                   # node tiles per prep DMA group
Q_ROWS = NWIN * WIN              # 6272
SB_WINDOWS = 3                   # windows per gather super-batch

BF16 = ml_dtypes.bfloat16
_dt = mybir.dt


def _pack_idx(idx: np.ndarray) -> np.ndarray:
    """[n] -> [128, n/16] int16 (stripe-of-16 column-major, replicated x8)."""
    n = idx.shape[0]
    assert n % 16 == 0
    t16 = idx.astype(np.int16).reshape(n // 16, 16).T
    return np.tile(t16, (8, 1))


def _host_prep(h, src, dst):
    """Per-core edge layout. Returns static plan + per-core arrays."""
    core_of = dst // NPC
    percore = []
    for c in range(CORES):
        sel = np.nonzero(core_of == c)[0]
        e_src = src[sel]
        e_dst = dst[sel] - c * NPC
        order = np.argsort(e_dst, kind="stable")
        e_src = e_src[order]
        e_dst = e_dst[order]
        own_lo = c * NPC
        pos = np.empty(N_NODES, np.int64)
        own = np.arange(own_lo, own_lo + NPC)
        others = np.concatenate([np.arange(0, own_lo), np.arange(own_lo + NPC, N_NODES)])
        perm = np.concatenate([own, others])        # table row r holds node perm[r]
        pos[perm] = np.arange(N_NODES)
        src_p = pos[e_src]
        w = e_dst // WIN
        is_lo = src_p < SPLIT
        percore.append(dict(src_p=src_p, e_dst=e_dst, w=w, is_lo=is_lo, perm=perm))

    T_lo = np.zeros(NWIN, np.int64)
    T_hi = np.zeros(NWIN, np.int64)
    for c in range(CORES):
        pc = percore[c]
        for w in range(NWIN):
            m = pc["w"] == w
            nlo = int((m & pc["is_lo"]).sum())
            nhi = int((m & ~pc["is_lo"]).sum())
            T_lo[w] = max(T_lo[w], (nlo + 127) // 128)
            T_hi[w] = max(T_hi[w], (nhi + 127) // 128)
    T_lo = np.maximum(T_lo, 1)
    T_hi = np.maximum(T_hi, 1)

    TT = int((T_lo + T_hi).sum())
    LO_TOT = int(T_lo.sum()) * 128
    HI_TOT = int(T_hi.sum()) * 128

    arrs = []
    for c in range(CORES):
        pc = percore[c]
        ilo = np.zeros(LO_TOT, np.int64)
        ihi = np.zeros(HI_TOT, np.int64)
        oh = np.zeros((128, TT * 128), dtype=np.int8)
        ohT = np.zeros((128, TT * 128), dtype=np.int8)
        lo_off = 0
        hi_off = 0
        proc = 0
        for w in range(NWIN):
            m = pc["w"] == w
            for cls in (0, 1):
                if cls == 0:
                    sel = np.nonzero(m & pc["is_lo"])[0]
                    ntile = int(T_lo[w])
                    vals = pc["src_p"][sel]
                else:
                    sel = np.nonzero(m & ~pc["is_lo"])[0]
                    ntile = int(T_hi[w])
                    vals = pc["src_p"][sel] - SPLIT
                cnt = sel.shape[0]
                assert ntile * 128 - cnt >= 0
                if cls == 0:
                    ilo[lo_off:lo_off + cnt] = vals
                    lo_off += ntile * 128
                else:
                    ihi[hi_off:hi_off + cnt] = vals
                    hi_off += ntile * 128
                dstrel = pc["e_dst"][sel] - w * WIN
                slot = np.arange(cnt)
                tile_i = proc + slot // 128
                oh[slot % 128, tile_i * 128 + dstrel] = 1
                ohT[dstrel, tile_i * 128 + slot % 128] = 1
                proc += ntile
        assert proc == TT
        arrs.append(dict(
            ilo=_pack_idx(ilo), ihi=_pack_idx(ihi), oh=oh, ohT=ohT,
            perm=pc["perm"],
        ))
    return dict(T_lo=T_lo, T_hi=T_hi, TT=TT, LO_TOT=LO_TOT, HI_TOT=HI_TOT), arrs


def _build_program(plan):
    STAGE = os.environ.get("KSTAGE", "full")
    T_lo, T_hi, TT = plan["T_lo"], plan["T_hi"], plan["TT"]
    LO_TOT, HI_TOT = plan["LO_TOT"], plan["HI_TOT"]

    nc = bacc.Bacc("TRN2", target_bir_lowering=False, debug=False, num_swdge_queues=4)
    hT = nc.dram_tensor("hT", [128, N_NODES], _dt.bfloat16, kind="ExternalInput")
    wkvq = nc.dram_tensor("wkvq", [128, KVQ_W], _dt.bfloat16, kind="ExternalInput")
    bqb = nc.dram_tensor("bqb", [128, F], _dt.bfloat16, kind="ExternalInput")
    bkb = nc.dram_tensor("bkb", [128, F], _dt.bfloat16, kind="ExternalInput")
    bvb = nc.dram_tensor("bvb", [128, F], _dt.bfloat16, kind="ExternalInput")
    ilo = nc.dram_tensor("ilo", [128, LO_TOT // 16], _dt.int16, kind="ExternalInput")
    ihi = nc.dram_tensor("ihi", [128, HI_TOT // 16], _dt.int16, kind="ExternalInput")
    oh = nc.dram_tensor("oh", [128, TT * 128], _dt.int8, kind="ExternalInput")
    ohT = nc.dram_tensor("ohT", [128, TT * 128], _dt.int8, kind="ExternalInput")
    out = nc.dram_tensor("out", [Q_ROWS, F], _dt.float32, kind="ExternalOutput")

    sbs = []
    w0 = 0
    while w0 < NWIN:
        sbs.append(list(range(w0, min(w0 + SB_WINDOWS, NWIN))))
        w0 += SB_WINDOWS

    with tile.TileContext(nc) as tc:
        with (
            tc.tile_pool(name="const", bufs=1) as constp,
            tc.tile_pool(name="dram", bufs=1, space="DRAM") as dramp,
            tc.tile_pool(name="prep_in", bufs=3) as prep_in,
            tc.tile_pool(name="prep_ps", bufs=2, space="PSUM") as prep_ps,
            tc.tile_pool(name="prep_out", bufs=3) as prep_out,
            tc.tile_pool(name="gath", bufs=2) as gath,
            tc.tile_pool(name="work", bufs=5) as work,
            tc.tile_pool(name="mps", bufs=2, space="PSUM") as mps,
            tc.tile_pool(name="fin", bufs=3) as finp,
        ):
            nc.gpsimd.load_library(library_config.mlp)

            wk_t = constp.tile([128, KVQ_W], _dt.bfloat16)
            nc.sync.dma_start(wk_t[:], wkvq[:, :])
            bq_t = constp.tile([128, F], _dt.bfloat16)
            nc.sync.dma_start(bq_t[:], bqb[:, :])
            bk_t = constp.tile([128, F], _dt.bfloat16)
            nc.sync.dma_start(bk_t[:], bkb[:, :])
            bv_t = constp.tile([128, F], _dt.bfloat16)
            nc.sync.dma_start(bv_t[:], bvb[:, :])

            kvq = dramp.tile([PREP_TILES * 128, KVQ_W], _dt.bfloat16)

            # ---------------- prep: KVQ table ----------------
            g0 = 0
            while g0 < PREP_TILES:
                g1 = min(g0 + PREP_GROUP, PREP_TILES)
                ng = g1 - g0
                r0 = g0 * 128
                ncol = min(ng * 128, N_NODES - r0) if r0 < N_NODES else 0
                htg = prep_in.tile([128, PREP_GROUP * 128], _dt.bfloat16, tag="htg")
                nc.sync.dma_start(htg[:, :ncol], hT[:, r0:r0 + ncol])
                kvg = prep_out.tile([128, PREP_GROUP, KVQ_W], _dt.bfloat16, tag="kvg")
                for j0 in range(0, ng, 2):
                    j1 = min(j0 + 2, ng)
                    # [128, 2, 512]: each 512-fp32 slot is one PSUM bank
                    ps = prep_ps.tile([128, 2, 512], _dt.float32, space="PSUM", tag="pps")
                    for j in range(j0, j1):
                        nc.tensor.matmul(
                            ps[:, j - j0, 0:KVQ_W],
                            lhsT=htg[:, j * 128:(j + 1) * 128], rhs=wk_t[:],
                            start=True, stop=True,
                        )
                    if (j0 // 2) % 2 == 0:
                        nc.scalar.copy(kvg[:, j0:j1, :], ps[:, 0:j1 - j0, 0:KVQ_W])
                    else:
                        nc.vector.tensor_copy(kvg[:, j0:j1, :], ps[:, 0:j1 - j0, 0:KVQ_W])
                nc.scalar.dma_start(
                    kvq[r0:r0 + ng * 128, :].rearrange("(t p) f -> p t f", p=128),
                    kvg[:, :ng, :],
                )
                g0 = g1

            # ---------------- main ----------------
            if STAGE == "prep":
                probe = finp.tile([128, F], _dt.float32, tag="probe")
                ksl = finp.tile([128, F], _dt.bfloat16, tag="ksl")
                nc.sync.dma_start(ksl[:], kvq[0:128, 0:F])
                nc.vector.tensor_copy(probe[:], ksl[:])
                nc.sync.dma_start(out[0:128, :], probe[:])
                sbs = []

            lo_pos = 0
            hi_pos = 0
            proc = 0
            for sb in sbs:
                nlo = int(sum(T_lo[w] for w in sb))
                nhi = int(sum(T_hi[w] for w in sb))
                nt = nlo + nhi
                ilo_t = gath.tile([128, nlo * 8], _dt.int16, tag="ilo")
                nc.sync.dma_start(ilo_t[:], ilo[:, lo_pos * 8:(lo_pos + nlo) * 8])
                ihi_t = gath.tile([128, nhi * 8], _dt.int16, tag="ihi")
                nc.sync.dma_start(ihi_t[:], ihi[:, hi_pos * 8:(hi_pos + nhi) * 8])
                oh_t = gath.tile([128, nt * 128], _dt.bfloat16, tag="oh")
                nc.gpsimd.dma_start(oh_t[:], oh[:, proc * 128:(proc + nt) * 128])
                ohT_t = gath.tile([128, nt * 128], _dt.bfloat16, tag="ohT")
                nc.gpsimd.dma_start(ohT_t[:], ohT[:, proc * 128:(proc + nt) * 128])

                kvlo = gath.tile([128, nlo, 2 * F], _dt.bfloat16, tag="kvlo")
                kvhi = gath.tile([128, nhi, 2 * F], _dt.bfloat16, tag="kvhi")
                # queue-striped gathers: 4 SWDGE queues run on distinct Q7
                # pairs and overlap their descriptor generation.
                for buf, idxt, n_t, base in (
                    (kvlo, ilo_t, nlo, 0),
                    (kvhi, ihi_t, nhi, SPLIT),
                ):
                    table = kvq[base:SPLIT if base == 0 else N_NODES, 0:2 * F]
                    step = (n_t + 3) // 4
                    t0 = 0
                    q = 0
                    while t0 < n_t:
                        t1 = min(t0 + step, n_t)
                        nsub = t1 - t0
                        nc.gpsimd.dma_gather(
                            buf[:, t0:t1, :], table, idxt[:, t0 * 8:t1 * 8],
                            nsub * 128, nsub * 128, 2 * F,
                            elem_step=KVQ_W, single_packet=False, queue_num=q,
                        )
                        t0 = t1
                        q = (q + 1) % 4

                sb_lo = 0
                sb_hi = 0
                sb_proc = 0
                for w in sb:
                    qwt = work.tile([128, F + H], _dt.bfloat16, tag="qw")
                    nc.scalar.dma_start(qwt[:, 0:F], kvq[w * WIN:(w + 1) * WIN, 2 * F:3 * F])
                    nc.vector.tensor_tensor(
                        out=qwt[:, 0:F], in0=qwt[:, 0:F], in1=bq_t[:],
                        op=mybir.AluOpType.add,
                    )
                    t1p = finp.tile([128, F], _dt.bfloat16, tag="t1p")
                    nc.vector.tensor_tensor(
                        out=t1p[:], in0=qwt[:, 0:F], in1=bk_t[:],
                        op=mybir.AluOpType.mult,
                    )
                    t1f = finp.tile([128, H], _dt.float32, tag="t1f")
                    nc.vector.tensor_reduce(
                        out=t1f[:],
                        in_=t1p[:].rearrange("p (h d) -> p h d", h=H, d=D),
                        axis=mybir.AxisListType.X,
                        op=mybir.AluOpType.add,
                    )
                    nc.scalar.copy(qwt[:, F:F + H], t1f[:])
                    outz_ps = mps.tile([128, F + H], _dt.float32, space="PSUM", tag="outz")
                    wtiles = int(T_lo[w] + T_hi[w])
                    kdone = 0
                    for cls in (0, 1):
                        tc_n = int(T_lo[w]) if cls == 0 else int(T_hi[w])
                        if tc_n == 0:
                            continue
                        if cls == 0:
                            kv_g, kpos = kvlo, sb_lo
                            sb_lo += tc_n
                        else:
                            kv_g, kpos = kvhi, sb_hi
                            sb_hi += tc_n
                        K_ap = kv_g[:, kpos:kpos + tc_n, 0:F]
                        V_ap = kv_g[:, kpos:kpos + tc_n, F:2 * F]

                        # Q expansion: per tile one-hot^T @ Q_win; 4 tiles
                        # share one PSUM bank and one batched evacuation.
                        qe = work.tile([128, tc_n, F + H], _dt.bfloat16, tag="qe")
                        k0 = 0
                        while k0 < tc_n:
                            k1 = min(k0 + 3, tc_n)
                            qe_ps = mps.tile([128, 3, F + H], _dt.float32, space="PSUM", tag="qeps")
                            for k in range(k0, k1):
                                nc.tensor.matmul(
                                    qe_ps[:, k - k0, :],
                                    lhsT=ohT_t[:, (sb_proc + k) * 128:(sb_proc + k + 1) * 128],
                                    rhs=qwt[:],
                                    start=True, stop=True,
                                )
                            nc.scalar.copy(qe[:, k0:k1, :], qe_ps[:, 0:k1 - k0, :])
                            k0 = k1

                        kq = work.tile([128, tc_n, F], _dt.bfloat16, tag="kq")
                        nc.vector.tensor_tensor(
                            out=kq[:], in0=K_ap, in1=qe[:, :, 0:F], op=mybir.AluOpType.mult
                        )
                        raw = work.tile([128, tc_n * H], _dt.float32, tag="raw")
                        nc.vector.tensor_reduce(
                            out=raw[:],
                            in_=kq[:].rearrange("p t (h d) -> p (t h) d", h=H, d=D),
                            axis=mybir.AxisListType.X,
                            op=mybir.AluOpType.add,
                        )
                        nc.vector.tensor_tensor(
                            out=raw[:].rearrange("p (t h) -> p t h", h=H),
                            in0=raw[:].rearrange("p (t h) -> p t h", h=H),
                            in1=qe[:, :, F:F + H],
                            op=mybir.AluOpType.add,
                        )
                        nc.vector.tensor_scalar(
                            out=raw[:], in0=raw[:],
                            scalar1=20.0, scalar2=-20.0,
                            op0=mybir.AluOpType.min, op1=mybir.AluOpType.max,
                        )
                        wv_s = work.tile([128, tc_n, F + H], _dt.bfloat16, tag="wvs")
                        nc.scalar.activation(
                            wv_s[:, :, F:F + H],
                            raw[:].rearrange("p (t h) -> p t h", h=H),
                            mybir.ActivationFunctionType.Exp, scale=0.25,
                        )
                        sbc = work.tile([128, tc_n, F], _dt.bfloat16, tag="sbc")
                        s_base = wv_s[:, 0:tc_n, F:F + H]
                        s_b = bass.AP(
                            s_base.tensor, s_base.offset,
                            [s_base.ap[0], [F + H, tc_n], [1, H], [0, D]],
                        )
                        nc.scalar.activation(
                            sbc[:].rearrange("p t (g d) -> p t g d", d=D),
                            s_b,
                            mybir.ActivationFunctionType.Copy,
                        )
                        nc.vector.tensor_tensor(
                            out=wv_s[:, :, 0:F],
                            in0=V_ap,
                            in1=sbc[:],
                            op=mybir.AluOpType.mult,
                        )
                        for k in range(tc_n):
                            nc.tensor.matmul(
                                outz_ps[:],
                                lhsT=oh_t[:, (sb_proc + k) * 128:(sb_proc + k + 1) * 128],
                                rhs=wv_s[:, k, :],
                                start=(kdone == 0), stop=(kdone == wtiles - 1),
                            )
                            kdone += 1
                        sb_proc += tc_n
                    # finalize window
                    zf = finp.tile([128, H], _dt.float32, tag="zf")
                    nc.vector.tensor_scalar_add(
                        out=zf[:], in0=outz_ps[:, F:F + H], scalar1=1e-6
                    )
                    rz = finp.tile([128, H], _dt.float32, tag="rz")
                    nc.vector.reciprocal(rz[:], zf[:])
                    fin = finp.tile([128, F], _dt.float32, tag="fin")
                    zb = bass.AP(zf.tensor, zf[:].offset, [zf[:].ap[0], [1, H], [0, D]])
                    term = finp.tile([128, F], _dt.float32, tag="term")
                    nc.vector.tensor_tensor(
                        out=term[:].rearrange("p (h d) -> p h d", h=H, d=D),
                        in0=bv_t[:].rearrange("p (h d) -> p h d", h=H, d=D),
                        in1=zb,
                        op=mybir.AluOpType.mult,
                    )
                    nc.vector.tensor_tensor(
                        out=term[:], in0=term[:], in1=outz_ps[:, 0:F],
                        op=mybir.AluOpType.add,
                    )
                    rz_b = bass.AP(rz.tensor, rz[:].offset, [rz[:].ap[0], [1, H], [0, D]])
                    nc.vector.tensor_tensor(
                        out=fin[:].rearrange("p (h d) -> p h d", h=H, d=D),
                        in0=term[:].rearrange("p (h d) -> p h d", h=H, d=D),
                        in1=rz_b,
                        op=mybir.AluOpType.mult,
                    )
                    nvalid = min(WIN, NPC - w * WIN)
                    nc.sync.dma_start(out[w * WIN:w * WIN + nvalid, :], fin[:nvalid, :])
                lo_pos += nlo
                hi_pos += nhi
                proc += nt

    nc.compile()
    return nc


def kernel(**inputs):
    h = np.asarray(inputs["h"], np.float32)
    src = np.asarray(inputs["src"]).astype(np.int64)
    dst = np.asarray(inputs["dst"]).astype(np.int64)
    Wq = np.asarray(inputs["Wq"], np.float32)
    bq = np.asarray(inputs["bq"], np.float32)
    Wk = np.asarray(inputs["Wk"], np.float32)
    bk = np.asarray(inputs["bk"], np.float32)
    Wv = np.asarray(inputs["Wv"], np.float32)
    bv = np.asarray(inputs["bv"], np.float32)

    plan, arrs = _host_prep(h, src, dst)
    nc = _build_program(plan)

    wkvq = np.concatenate([Wk, Wv, Wq], axis=1).astype(BF16)
    bqb = np.ascontiguousarray(np.broadcast_to(bq, (128, F))).astype(BF16)
    bkb = np.ascontiguousarray(np.broadcast_to(bk, (128, F))).astype(BF16)
    bvb = np.ascontiguousarray(np.broadcast_to(bv, (128, F))).astype(BF16)

    in_maps = []
    for c in range(CORES):
        a = arrs[c]
        hTc = np.ascontiguousarray(h[a["perm"], :].T).astype(BF16)
        in_maps.append({
            "hT": hTc,
            "wkvq": wkvq,
            "bqb": bqb,
            "bkb": bkb,
            "bvb": bvb,
            "ilo": a["ilo"],
            "ihi": a["ihi"],
            "oh": a["oh"],
            "ohT": a["ohT"],
        })

    res = run_bass_kernel_spmd(nc, in_maps, core_ids=list(range(CORES)))
    out = np.concatenate(
        [res.results[c]["out"][:NPC] for c in range(CORES)], axis=0
    )
    return out.reshape(N_NODES, H, D)
